# revision 10
# baseline (speedup 1.0000x reference)
"""Bass/Trainium2 kernel for nn_BiDirectionalCrossAttentionLayer.

Sharding: 8 cores = batch(4) x head-group(2). Each core computes, for its
batch b and its 4 heads, the full 4-stream cross-attention + the 256 output
rows (t = hg*256 .. hg*256+255) of every stream. The reference's
"transpose(1,2) ... transpose/reshape" scramble maps output row t to
(head t//64, head-dim t%64) over all sequence positions, so a head-split of
attention is exactly an output-row split of everything after it.

All matmuls in bf16 (fp32 accumulate); residuals/LN in fp32.
"""

import os
import numpy as np
import ml_dtypes

import concourse.bacc as bacc
import concourse.bass as bass
import concourse.tile as tile
from concourse import mybir
from concourse.bass_utils import run_bass_kernel_spmd
from concourse.masks import make_identity

BF16 = ml_dtypes.bfloat16
F32 = np.float32

NS, B, S, E, H, HD = 4, 4, 512, 512, 8, 64
SCALE = HD ** -0.5
LN_EPS = 1e-5
P = 128
HG = 2            # head groups == cores per batch
HPC = H // HG // 2  # head-pairs per core = 2
HC = H // HG      # heads per core = 4
TG = S // HG      # output rows per core per stream = 256
TS = TG // P      # row tiles per core = 2
ET = E // P       # embedding tiles = 4
KT = S // P       # key/seq tiles = 4
FT = 4 * E // P   # ffn hidden tiles = 16
N_CORES = B * HG

AF = mybir.ActivationFunctionType
ALU = mybir.AluOpType
AX = mybir.AxisListType
DT_BF = mybir.dt.bfloat16
DT_F32 = mybir.dt.float32
DT_F8 = mybir.dt.float8e4
F8 = mybir.dt.np(mybir.dt.float8e4)
DR = mybir.MatmulPerfMode.DoubleRow
W_SCALE = 64.0  # host-side W1/W2 prescale: lifts fp8e4 out of subnormals


def _build_program(reps=1, phases="all"):
    nc = bacc.Bacc("TRN2", target_bir_lowering=False, debug=False)

    def din(name, shape, dt=DT_BF):
        return nc.dram_tensor(name, list(shape), dt, kind="ExternalInput").ap()

    xT_d = din("xT", (NS, P, ET, S))            # xT[n,p,et,s] = x[n,b,s,et*128+p]
    x32_d = din("x32", (NS, P, TS, E), DT_F32)  # x rows t-slice
    wq_d = din("wq", (NS, P, ET, HC * HD))      # Wq[n, e, hg*256 + c]
    wk_d = din("wk", (NS, P, ET, HC * HD))
    wv_d = din("wv", (NS, P, ET, HC * HD))
    wo_d = din("wo", (NS, P, ET, E))            # Wo[n]/NS, rows e
    w1_d = din("w1", (NS, P, ET, 4 * E), DT_F8)
    w2_d = din("w2", (NS, P, FT, E), DT_F8)
    cmat_d = din("cmat", (P, NS * NS), DT_F32)  # SCALE*inter broadcast on p
    g1_d = din("g1", (NS, E), DT_F32)
    b1_d = din("b1", (NS, E), DT_F32)
    g2_d = din("g2", (NS, E), DT_F32)
    b2_d = din("b2", (NS, E), DT_F32)
    bf1_d = din("bf1", (NS, 4 * E))             # bf1 row (K=1 matmul operand)
    bf2_d = din("bf2", (NS, E), DT_F32)
    out_d = nc.dram_tensor("out", [NS, P, TS, E], DT_F32, kind="ExternalOutput").ap()

    with tile.TileContext(nc) as tc:
        with tc.tile_pool(name="const", bufs=1) as const:
            identf = const.tile([P, P], DT_F32)
            make_identity(nc, identf[:])
            cmat_sb = const.tile([P, NS * NS], DT_F32)
            nc.sync.dma_start(cmat_sb[:], cmat_d[:])
            eps_sb = const.tile([P, 1], DT_F32)
            nc.gpsimd.memset(eps_sb[:], LN_EPS)

            # long-lived activations
            r1 = const.tile([P, NS, TS, E], DT_F32)
            r1T = const.tile([P, NS, ET, TG], DT_F8)

          # replicated body via HW loop (reps>1 only for slope timing)
          # fmt: off
            import contextlib
            _loop = tc.For_i(0, reps, 1) if reps > 1 else contextlib.nullcontext()
            with _loop:
              # FFN weights (fp8) loaded upfront: their DMAs have no
              # deps, so they prefetch during the DMA-idle attention phase.
              f_w1 = tc.alloc_tile_pool(name="f_w1", bufs=1)
              w1all = f_w1.tile([P, NS, ET, 4 * E], DT_F8)
              scopeB = tc.alloc_tile_pool(name="scopeB", bufs=1)
              x32 = scopeB.tile([P, NS, TS, E], DT_F32)
              att = scopeB.tile([P, NS, KT, HC * HD], DT_BF)  # att_std accum
              nc.gpsimd.memset(att[:], 0.0)
              for n in range(NS):
                  nc.sync.dma_start(x32[:, n], x32_d[n])

              c_w = tc.alloc_tile_pool(name="c_w", bufs=1)
              g1b = c_w.tile([P, NS, E], DT_F32)
              b1b = c_w.tile([P, NS, E], DT_F32)
              for n in range(NS):
                  nc.sync.dma_start(g1b[:, n], g1_d[n].partition_broadcast(P))
                  nc.sync.dma_start(b1b[:, n], b1_d[n].partition_broadcast(P))
              scopeA = tc.alloc_tile_pool(name="scopeA", bufs=1)
              qT = scopeA.tile([P, NS, HPC, S], DT_BF)   # [d-pair rows, n, hp, q]
              kT = scopeA.tile([P, NS, HPC, S], DT_BF)
              # inner dim padded to 80B: DoubleRow ldweights needs the
              # kt-pair stride (HC*inner) and head offset 16B-aligned.
              # cols HD+1..79 are zeroed so attn@v can emit 80 output rows
              # (a 16-multiple, required by the xbar DMA transpose).
              vex = scopeA.tile([P, NS, KT, HC, 80], DT_F8)
              nc.gpsimd.memset(vex[:, :, :, :, HD:HD + 1], 1.0)
              # cols HD+1..79 zeroed -> attn@v emits exact-zero rows 65..79,
              # padding uas to 80 rows (16-multiple for the xbar transpose)
              nc.gpsimd.memset(vex[:, :, :, :, HD + 1:], 0.0)

              # Attention pools allocated BEFORE phase-1's so their SBUF/PSUM
              # space is disjoint (no released-zone reuse dep): the j-loop can
              # then overlap the tail of the QKV projections.
              a_sps = tc.alloc_tile_pool(name="a_sps", bufs=2, space="PSUM")
              a_ups = tc.alloc_tile_pool(name="a_ups", bufs=2, space="PSUM")
              a_sb = tc.alloc_tile_pool(name="a_sb", bufs=6)
              a_ub = tc.alloc_tile_pool(name="a_ub", bufs=3)
              a_sm = tc.alloc_tile_pool(name="a_sm", bufs=16)
              a_tr = tc.alloc_tile_pool(name="a_tr", bufs=2)

              # ---------------- Phase 1: QKV projections ----------------
              # Emitted in an order that unblocks attention j=0 early:
              # k0, v0, q0..q3, then k/v for streams 1-3 (these overlap the
              # attention loop). Single-slot psum: chains serialize on the
              # copy, but the attention stream fills the PE gaps.
              p1w = tc.alloc_tile_pool(name="p1w", bufs=1)
              # phase-1 chain psums share the transpose pool's two slots
              # (same tag): phase-1 gets double-buffering, and once it
              # drains the attention transposes inherit both banks.
              a_tps = tc.alloc_tile_pool(name="a_tps", bufs=2, space="PSUM")
              xTs = p1w.tile([P, NS, ET, S], DT_BF)
              wqs = p1w.tile([P, NS, ET, HC * HD], DT_BF)
              wks = p1w.tile([P, NS, ET, HC * HD], DT_BF)
              wvs = p1w.tile([P, NS, ET, HC * HD], DT_BF)
              for n in range(NS):
                  nc.sync.dma_start(xTs[:, n], xT_d[n])
                  nc.sync.dma_start(wqs[:, n], wq_d[n])
                  nc.sync.dma_start(wks[:, n], wk_d[n])
                  nc.sync.dma_start(wvs[:, n], wv_d[n])

              def p1_q(n):
                  for hp in range(HPC):
                      ps_q = a_tps.tile([P, S], DT_F32, tag="tr")
                      for et in range(ET):
                          nc.tensor.matmul(
                              ps_q[:], wqs[:, n, et, hp * P:(hp + 1) * P],
                              xTs[:, n, et], start=(et == 0),
                              stop=(et == ET - 1))
                      nc.vector.tensor_copy(qT[:, n, hp], ps_q[:])

              def p1_k(n):
                  for hp in range(HPC):
                      ps_k = a_tps.tile([P, S], DT_F32, tag="tr")
                      for et in range(ET):
                          nc.tensor.matmul(
                              ps_k[:], wks[:, n, et, hp * P:(hp + 1) * P],
                              xTs[:, n, et], start=(et == 0),
                              stop=(et == ET - 1))
                      nc.vector.tensor_copy(kT[:, n, hp], ps_k[:])

              def p1_v(n):
                  for kt in range(KT):
                      ps_v = a_tps.tile([P, S], DT_F32, tag="tr")
                      for et in range(ET):
                          nc.tensor.matmul(
                              ps_v[:, 0:HC * HD],
                              xTs[:, n, et, kt * P:(kt + 1) * P],
                              wvs[:, n, et], start=(et == 0),
                              stop=(et == ET - 1))
                      nc.vector.tensor_copy(
                          vex[:, n, kt, :, 0:HD],
                          ps_v[:, 0:HC * HD].rearrange("p (h d) -> p h d",
                                                       d=HD))

              p1_k(0), p1_v(0)
              for n in range(NS):
                  p1_q(n)
              for n in range(1, NS):
                  p1_k(n), p1_v(n)

              # ---------------- Phase 2: cross-stream attention ----------------
              # Heads of a pair live on disjoint PE row strips (partitions
              # 0-63 / 64-127): issuing their score matmuls back-to-back lets
              # the PE run them concurrently (implicit tile_position row
              # packing from the operands' base partitions).
              if True:
                  def wo_ln1(i, wo_n, c_ps, c_tp, c_sb, c_sm):
                      # Wo proj + residual + LayerNorm1 + r1 transpose for
                      # stream i. invstd = exp(-0.5*ln(var/E + eps)) keeps ACT
                      # on the ln/exp table set shared with attention's exps.
                      for ts in range(TS):
                          wo_ps = c_ps.tile([P, E], DT_F32, tag="wops")
                          for qt in range(KT):
                              nc.tensor.matmul(
                                  wo_ps[:], att[:, i, qt, ts * P:(ts + 1) * P],
                                  wo_n[:, qt], start=(qt == 0),
                                  stop=(qt == KT - 1))
                          y1 = c_sb.tile([P, E], DT_F32, tag="y1")
                          nc.vector.tensor_add(y1[:], wo_ps[:], x32[:, i, ts])
                          nm = c_sm.tile([P, 1], DT_F32, tag="nm")
                          nc.vector.reduce_sum(nm[:], y1[:], axis=AX.X)
                          nc.vector.tensor_scalar_mul(nm[:], nm[:], -1.0 / E)
                          xc = c_sb.tile([P, E], DT_F32, tag="xc")
                          nc.vector.tensor_scalar_add(xc[:], y1[:], nm[:])
                          var = c_sm.tile([P, 1], DT_F32, tag="var")
                          sq = c_sb.tile([P, E], DT_F32, tag="sq")
                          nc.vector.scalar_tensor_tensor(
                              out=sq[:], in0=xc[:], scalar=1.0, in1=xc[:],
                              op0=ALU.mult, op1=ALU.mult, accum_out=var[:])
                          inv = c_sm.tile([P, 1], DT_F32, tag="inv")
                          # Sqrt + DVE reciprocal: Sqrt-only keeps ACT on one
                          # table set for the whole phase (Ln/Exp alternation
                          # forced a 1.3us table reload per LN).
                          nc.scalar.activation(inv[:], var[:], AF.Sqrt,
                                               bias=eps_sb[:], scale=1.0 / E)
                          nc.vector.reciprocal(inv[:], inv[:])
                          nc.vector.scalar_tensor_tensor(
                              out=r1[:, i, ts], in0=xc[:], scalar=inv[:],
                              in1=g1b[:, i], op0=ALU.mult, op1=ALU.mult)
                          nc.vector.tensor_add(r1[:, i, ts], r1[:, i, ts],
                                               b1b[:, i])
                          for et in range(ET):
                              rt_ps = c_tp.tile([P, P], DT_F32, tag="rt")
                              nc.tensor.transpose(
                                  rt_ps[:], r1[:, i, ts, et * P:(et + 1) * P],
                                  identf[:])
                              nc.vector.tensor_copy(
                                  r1T[:, i, et, ts * P:(ts + 1) * P], rt_ps[:])

                  # j-major: per key-stream j, do scores+exp for a pair of
                  # query-streams g={i0,i1}, then attn@v with the vex
                  # stationary shared by consecutive matmuls (ldweights
                  # elision). The [d,q]->[q,d] flip of the attn@v outputs
                  # rides the xbar DMA-transpose engine (idle) instead of
                  # the PE: evac psum->sbuf once, DMA-transpose per i2 into
                  # trT[q, i2, qt, hl, d], then one batched reciprocal and
                  # one fused accumulate per (i, qt).
                  attn_on = phases in ("all", "attn", "mmx", "mm2")
                  act_on = phases in ("all", "attn")
                  norm_on = phases != "mm2"
                  for j in range(NS if attn_on else 0):
                      for g in range(2):
                          trT = a_tr.tile([P, 2, KT, HC, 80], DT_BF,
                                          tag="trT")
                          for hp in range(HPC):
                              exs = []  # per i2: ex tile [P, 2, KT, S]
                              for i2 in range(2):
                                  i = g * 2 + i2
                                  c_ap = cmat_sb[:,
                                                 (i * NS + j):(i * NS + j + 1)]
                                  ex = a_sb.tile([P, 2, KT, S], DT_F8,
                                                 tag="ex")
                                  for kt in range(KT):
                                      s = a_sps.tile([P, 2, S], DT_F32,
                                                     tag="s")
                                      nc.tensor.matmul(
                                          s[:, 0],
                                          kT[0:HD, j, hp, kt * P:(kt + 1) * P],
                                          qT[0:HD, i, hp],
                                          start=True, stop=True)
                                      nc.tensor.matmul(
                                          s[:, 1],
                                          kT[HD:P, j, hp, kt * P:(kt + 1) * P],
                                          qT[HD:P, i, hp],
                                          start=True, stop=True)
                                      if act_on:
                                          nc.scalar.activation(
                                              ex[:, :, kt, :], s[:], AF.Exp,
                                              scale=c_ap)
                                      else:
                                          # timing knockout: keep the psum
                                          # live with a tiny ACT read
                                          nc.scalar.activation(
                                              ex[:, :, kt, 0:4], s[:, :, 0:4],
                                              AF.Exp, scale=c_ap)
                                  exs.append(ex)
                              for sub in range(2):
                                  hl = hp * 2 + sub
                                  uas = [a_ups.tile([80, S], DT_F32,
                                                    tag="ua", name=f"ua{i2}")
                                         for i2 in range(2)]
                                  for ktp in range(KT // 2):
                                      # same vex stationary for both i2 mms
                                      for i2 in range(2):
                                          nc.tensor.matmul(
                                              uas[i2][:],
                                              vex[:, j,
                                                  2 * ktp:2 * ktp + 2, hl,
                                                  0:80],
                                              exs[i2][:, sub,
                                                      2 * ktp:2 * ktp + 2],
                                              start=(ktp == 0),
                                              stop=(ktp == KT // 2 - 1),
                                              perf_mode=DR)
                                  usb = a_ub.tile([80, 2, S], DT_BF,
                                                  tag="usb")
                                  for i2 in range(2):
                                      nc.vector.tensor_copy(usb[:, i2],
                                                            uas[i2][:])
                                      nc.scalar.dma_start(
                                          trT[:, i2, :, hl, :], usb[:, i2],
                                          transpose=True)
                          for i2 in range(2 if norm_on else 0):
                              i = g * 2 + i2
                              rr = a_sm.tile([P, KT], DT_F32, tag="rr")
                              nc.vector.reciprocal(
                                  rr[:], trT[:, i2, :, 0, HD:HD + 1])
                              for qt in range(KT):
                                  att_ap = att[:, i, qt, :].rearrange(
                                      "p (h d) -> p h d", h=HC)
                                  nc.vector.scalar_tensor_tensor(
                                      out=att_ap,
                                      in0=trT[:, i2, qt, :, 0:HD],
                                      scalar=rr[:, qt:qt + 1],
                                      in1=att_ap,
                                      op0=ALU.mult, op1=ALU.add)
                          if not norm_on:
                              # keep trT live: fold into att via add
                              nc.vector.tensor_add(
                                  att[:, g].rearrange(
                                      "p a (h d) -> p a h d", h=HC),
                                  att[:, g].rearrange(
                                      "p a (h d) -> p a h d", h=HC),
                                  trT[:, 0, :, :, 0:HD])
                      # trickle one FFN W1 stream load per j iteration
                      nc.sync.dma_start(w1all[:, j], w1_d[j])

              if not attn_on and phases in ("noattn",):
                  # attention loop skipped: load FFN weights here instead
                  for n in range(NS):
                      nc.sync.dma_start(w1all[:, n], w1_d[n])

              if phases == "qkv":
                  # keep q/k/v live so DCE cannot drop the projections
                  with tc.tile_pool(name="qo", bufs=2) as qo:
                      for n in range(NS):
                          qout = qo.tile([P, TS, E], DT_F32, tag="qout")
                          nc.vector.tensor_copy(
                              qout[:].rearrange("p a b -> p (a b)"),
                              qT[:, n].rearrange("p a b -> p (a b)"))
                          nc.vector.tensor_add(
                              qout[:].rearrange("p a b -> p (a b)"),
                              qout[:].rearrange("p a b -> p (a b)"),
                              kT[:, n].rearrange("p a b -> p (a b)"))
                          nc.vector.tensor_add(
                              qout[:, 0, 0:256].rearrange(
                                  "p (a c) -> p a c", a=KT),
                              qout[:, 0, 0:256].rearrange(
                                  "p (a c) -> p a c", a=KT),
                              vex[:, n, :, 0, 0:HD])
                          nc.sync.dma_start(out_d[n], qout[:])

              a_tps.release(), p1w.release(), a_tr.release()
              a_sm.release(), a_ub.release(), a_sb.release()
              a_ups.release(), a_sps.release()
              scopeA.release()

              # -------- Phase 3: Wo proj + residual + LN1 -------------------
              if phases not in ("attn", "qkv", "mmx", "mm2"):
                  with tc.tile_pool(name="c_ps", bufs=2, space="PSUM") as c_ps, \
                       tc.tile_pool(name="c_tp", bufs=2, space="PSUM") as c_tp, \
                       tc.tile_pool(name="c_sb", bufs=3) as c_sb, \
                       tc.tile_pool(name="c_wo", bufs=2) as c_wo, \
                       tc.tile_pool(name="c_sm", bufs=6) as c_sm:
                      for i in range(NS):
                          wo_n = c_wo.tile([P, ET, E], DT_BF, tag="won")
                          nc.sync.dma_start(wo_n[:], wo_d[i])
                          wo_ln1(i, wo_n, c_ps, c_tp, c_sb, c_sm)

              if phases in ("attn", "mmx", "mm2"):
                  # copy att into the output so DCE cannot drop the attention
                  with tc.tile_pool(name="ao", bufs=2) as ao:
                      for n in range(NS):
                          aout = ao.tile([P, TS, E], DT_F32, tag="aout")
                          nc.vector.tensor_copy(
                              aout[:].rearrange("p a b -> p (a b)"),
                              att[:, n].rearrange("p a b -> p (a b)"))
                          nc.sync.dma_start(out_d[n], aout[:])
              c_w.release()
              scopeB.release()

              # ---------------- Phase 4: FFN W1 + gelu (all streams) ------
              # All gelu before any LN2 sqrt: one ACT table load per set.
              hT_pool = tc.alloc_tile_pool(name="hT_pool", bufs=1)
              hTall = hT_pool.tile([P, NS, FT, TG], DT_F8)
              f_c = tc.alloc_tile_pool(name="f_c", bufs=1)
              bf1r = f_c.tile([1, NS, 4 * E], DT_BF)
              ones_row = f_c.tile([1, TG], DT_BF)
              nc.gpsimd.memset(ones_row[:], 1.0)
              g2b = f_c.tile([P, NS, E], DT_F32)
              b2b = f_c.tile([P, NS, E], DT_F32)
              bf2b = f_c.tile([P, NS, E], DT_F32)
              nc.sync.dma_start(bf1r[:], bf1_d[None, :, :])
              for n in range(NS):
                  nc.sync.dma_start(g2b[:, n], g2_d[n].partition_broadcast(P))
                  nc.sync.dma_start(b2b[:, n], b2_d[n].partition_broadcast(P))
                  nc.sync.dma_start(bf2b[:, n], bf2_d[n].partition_broadcast(P))

              # bf1 enters via a K=1 ones-row matmul so gelu can batch
              # 4 hidden slices per ACTIVATE with no per-slice bias.
              with tc.tile_pool(name="f_ps", bufs=3, space="PSUM") as f_ps:
                  for n in range(NS if phases in ("all", "noattn") else 0):
                      for f4 in range(FT // 4):
                          h_ps = f_ps.tile([P, 4, TG], DT_F32, tag="hps")
                          for s4 in range(4):
                              fs = f4 * 4 + s4
                              for etp in range(ET // 2):
                                  nc.tensor.matmul(
                                      h_ps[:, s4],
                                      w1all[:, n, 2 * etp:2 * etp + 2,
                                            fs * P:(fs + 1) * P],
                                      r1T[:, n, 2 * etp:2 * etp + 2],
                                      start=(etp == 0), stop=False,
                                      perf_mode=DR)
                              nc.tensor.matmul(
                                  h_ps[:, s4], bf1r[0:1, n, fs * P:(fs + 1) * P],
                                  ones_row[:], start=False, stop=True)
                          # psum holds W_SCALE*(r1@W1 + bf1)
                          nc.scalar.activation(hTall[:, n, f4 * 4:(f4 + 1) * 4],
                                               h_ps[:], AF.Gelu,
                                               scale=1.0 / W_SCALE)

              # ---------------- Phase 5: FFN W2 + residual + LN2 ----------
              with tc.tile_pool(name="f_ps2", bufs=3, space="PSUM") as f_ps2, \
                   tc.tile_pool(name="f_sb", bufs=2) as f_sb, \
                   tc.tile_pool(name="f_sb2", bufs=3) as f_sb2, \
                   tc.tile_pool(name="f_w2", bufs=2) as f_w2, \
                   tc.tile_pool(name="f_sm", bufs=6) as f_sm:
                  for n in range(NS if phases in ("all", "noattn") else 0):
                      w2s = f_w2.tile([P, FT, E], DT_F8, tag="w2s")
                      nc.sync.dma_start(w2s[:], w2_d[n])
                      out_sb = f_sb.tile([P, TS, E], DT_F32, tag="outsb")
                      for ts in range(TS):
                          f2_ps = f_ps2.tile([P, E], DT_F32, tag="fps")
                          for ftp in range(FT // 2):
                              nc.tensor.matmul(
                                  f2_ps[:],
                                  hTall[:, n, 2 * ftp:2 * ftp + 2,
                                        ts * P:(ts + 1) * P],
                                  w2s[:, 2 * ftp:2 * ftp + 2],
                                  start=(ftp == 0), stop=(ftp == FT // 2 - 1),
                                  perf_mode=DR)
                          y2 = f_sb2.tile([P, E], DT_F32, tag="y2")
                          # psum holds W_SCALE*(h@W2): scale down + bias
                          nc.vector.scalar_tensor_tensor(
                              out=y2[:], in0=f2_ps[:], scalar=1.0 / W_SCALE,
                              in1=bf2b[:, n], op0=ALU.mult, op1=ALU.add)
                          nc.vector.tensor_add(y2[:], y2[:], r1[:, n, ts])
                          # LayerNorm 2
                          nm = f_sm.tile([P, 1], DT_F32, tag="nm2")
                          nc.vector.reduce_sum(nm[:], y2[:], axis=AX.X)
                          nc.vector.tensor_scalar_mul(nm[:], nm[:], -1.0 / E)
                          xc = f_sb2.tile([P, E], DT_F32, tag="xc2")
                          nc.vector.tensor_scalar_add(xc[:], y2[:], nm[:])
                          var = f_sm.tile([P, 1], DT_F32, tag="var2")
                          sq = f_sb2.tile([P, E], DT_F32, tag="sq2")
                          nc.vector.scalar_tensor_tensor(
                              out=sq[:], in0=xc[:], scalar=1.0, in1=xc[:],
                              op0=ALU.mult, op1=ALU.mult, accum_out=var[:])
                          inv = f_sm.tile([P, 1], DT_F32, tag="inv2")
                          nc.scalar.activation(inv[:], var[:], AF.Sqrt,
                                               bias=eps_sb[:], scale=1.0 / E)
                          nc.vector.reciprocal(inv[:], inv[:])
                          nc.vector.scalar_tensor_tensor(
                              out=out_sb[:, ts], in0=xc[:], scalar=inv[:],
                              in1=g2b[:, n], op0=ALU.mult, op1=ALU.mult)
                          nc.vector.tensor_add(out_sb[:, ts], out_sb[:, ts],
                                               b2b[:, n])
                      nc.sync.dma_start(out_d[n], out_sb[:])
              f_c.release()
              hT_pool.release()
              f_w1.release()

    nc.compile()
    return nc


_NC_CACHE = {}


def _get_nc(reps=1, phases="all"):
    key = f"nc{reps}_{phases}"
    if key not in _NC_CACHE:
        _NC_CACHE[key] = _build_program(reps, phases)
    return _NC_CACHE[key]


def _pack_inputs(x0, x1, x2, x3, Wq, Wk, Wv, Wo, bo, ln1_g, ln1_b, ln2_g, ln2_b,
                 W1, bf1, W2, bf2, inter):
    x = np.stack([np.asarray(x0), np.asarray(x1), np.asarray(x2),
                  np.asarray(x3)]).astype(F32)  # [NS,B,S,E]
    Wq, Wk, Wv, Wo = (np.asarray(a, F32) for a in (Wq, Wk, Wv, Wo))
    inputs_bo = np.asarray(bo, F32)
    W1, W2 = np.asarray(W1, F32), np.asarray(W2, F32)
    inter = np.asarray(inter, F32)

    def tile_rows(a, nt):
        # [NS, R, C] -> [NS, P, nt, C]
        return np.ascontiguousarray(
            a.reshape(NS, nt, P, a.shape[-1]).transpose(0, 2, 1, 3))

    shared = {
        "wo": tile_rows(Wo / NS, ET).astype(BF16),
        "w1": tile_rows(W1 * W_SCALE, ET).astype(F8),
        "w2": tile_rows(W2 * W_SCALE, FT).astype(F8),
        "cmat": np.ascontiguousarray(
            np.broadcast_to((inter * SCALE).reshape(1, NS * NS), (P, NS * NS))
        ).astype(F32),
        "g1": np.ascontiguousarray(ln1_g, dtype=F32),
        "b1": np.ascontiguousarray(ln1_b, dtype=F32),
        "g2": np.ascontiguousarray(ln2_g, dtype=F32),
        "b2": np.ascontiguousarray(ln2_b, dtype=F32),
        "bf1": np.ascontiguousarray(np.asarray(bf1, F32) * W_SCALE).astype(BF16),
        "bf2": np.ascontiguousarray(bf2, dtype=F32),
    }
    per_hg = []
    for hg in range(HG):
        cols = slice(hg * HC * HD, (hg + 1) * HC * HD)
        per_hg.append({
            "wq": tile_rows(Wq[:, :, cols], ET).astype(BF16),
            "wk": tile_rows(Wk[:, :, cols], ET).astype(BF16),
            "wv": tile_rows(Wv[:, :, cols], ET).astype(BF16),
        })
    in_maps = []
    for core in range(N_CORES):
        b, hg = core // HG, core % HG
        xb = x[:, b]  # [NS, S, E]
        xT = np.ascontiguousarray(
            xb.transpose(0, 2, 1).reshape(NS, ET, P, S).transpose(0, 2, 1, 3)
        ).astype(BF16)
        x32 = np.ascontiguousarray(
            (xb[:, hg * TG:(hg + 1) * TG] + np.asarray(
                inputs_bo)[:, None, :]).reshape(NS, TS, P, E)
            .transpose(0, 2, 1, 3).astype(F32))
        m = {"xT": xT, "x32": x32}
        m.update(shared)
        m.update(per_hg[hg])
        in_maps.append(m)
    return in_maps


def _unpack_outputs(results):
    full = np.empty((NS, B, S, E), dtype=F32)
    for core in range(N_CORES):
        b, hg = core // HG, core % HG
        o = results[core]["out"]  # [NS, P, TS, E]
        full[:, b, hg * TG:(hg + 1) * TG] = (
            o.transpose(0, 2, 1, 3).reshape(NS, TG, E))
    return tuple(full[n] for n in range(NS))


def kernel(**inputs):
    nc = _get_nc()
    in_maps = _pack_inputs(**inputs)
    res = run_bass_kernel_spmd(
        nc, in_maps, core_ids=list(range(N_CORES)),
        trace=bool(int(os.environ.get("KERNEL_TRACE", "0"))))
    _NC_CACHE["last_result"] = res
    return _unpack_outputs(res.results)


def _make_bench_fn(inputs, reps=1, phases="all"):
    """Build a jitted on-device executable for the kernel with `reps`
    replications of the body (HW loop). Returns (fn, concat_args)."""
    import jax
    import jax.numpy as jnp
    from jax.sharding import Mesh, PartitionSpec, NamedSharding
    from jax.experimental.shard_map import shard_map
    from concourse import bass2jax
    from concourse import mybir as mb

    nc = _get_nc(reps, phases)
    bass2jax.install_neuronx_cc_hook()
    in_maps = _pack_inputs(**inputs)

    part_name = nc.partition_id_tensor.name if nc.partition_id_tensor else None
    in_names, out_names, out_avals, zero_outs = [], [], [], []
    for alloc in nc.m.functions[0].allocations:
        if not isinstance(alloc, mb.MemoryLocationSet):
            continue
        name = alloc.memorylocations[0].name
        if alloc.kind == "ExternalInput":
            if name != part_name:
                in_names.append(name)
        elif alloc.kind == "ExternalOutput":
            out_names.append(name)
            shape = tuple(alloc.tensor_shape)
            dtype = mb.dt.np(alloc.dtype)
            out_avals.append(jax.core.ShapedArray(shape, dtype))
            zero_outs.append(np.zeros(shape, dtype))
    n_params = len(in_names)
    all_names = in_names + out_names
    if part_name is not None:
        all_names = all_names + [part_name]

    def _body(*args):
        operands = list(args)
        if part_name is not None:
            operands.append(bass2jax.partition_id_tensor())
        outs = bass2jax._bass_exec_p.bind(
            *operands, out_avals=tuple(out_avals), in_names=tuple(all_names),
            out_names=tuple(out_names), lowering_input_output_aliases=(),
            sim_require_finite=True, sim_require_nnan=True, nc=nc)
        return tuple(outs)

    devices = jax.devices()[:N_CORES]
    mesh = Mesh(np.asarray(devices), ("core",))
    spec = PartitionSpec("core")
    fn = jax.jit(shard_map(
        _body, mesh=mesh, in_specs=(spec,) * (n_params + len(out_names)),
        out_specs=(spec,) * len(out_names), check_rep=False))
    sh = NamedSharding(mesh, spec)
    concat = [jax.device_put(
        np.concatenate([in_maps[c][nm] for c in range(N_CORES)], axis=0), sh)
        for nm in in_names]
    concat += [jax.device_put(
        np.zeros((N_CORES * z.shape[0], *z.shape[1:]), z.dtype), sh)
        for z in zero_outs]

    out = fn(*concat)  # compile
    jax.block_until_ready(out)
    return fn, concat


def _time_fn(fn, concat):
    import time
    import jax
    t0 = time.perf_counter()
    out = fn(*concat)
    jax.block_until_ready(out)
    return time.perf_counter() - t0


def bench(inputs, iters=20, reps=1, phases="all"):
    """Time the on-device execution with device-resident inputs.
    Returns (min, median) seconds per call."""
    fn, concat = _make_bench_fn(inputs, reps, phases)
    times = sorted(_time_fn(fn, concat) for _ in range(iters))
    return times[0], times[len(times) // 2]


def bench_paired(inputs, iters=30, reps_hi=33, phases="all", phases_lo=None):
    """Robust per-rep device time: interleave a 1-rep and a reps_hi-rep
    executable; per-round difference cancels RPC/driver drift. Returns
    (median_slope_s, min_slope_s, raw_diffs)."""
    fn1, c1 = _make_bench_fn(inputs, 1, phases_lo or phases)
    fnH, cH = _make_bench_fn(inputs, reps_hi, phases)
    # warm both
    for _ in range(3):
        _time_fn(fn1, c1), _time_fn(fnH, cH)
    diffs = []
    for _ in range(iters):
        t1 = _time_fn(fn1, c1)
        tH = _time_fn(fnH, cH)
        t1b = _time_fn(fn1, c1)
        diffs.append(tH - min(t1, t1b))
    diffs.sort()
    n = reps_hi - 1
    return diffs[len(diffs) // 2] / n, diffs[0] / n, diffs


if __name__ == "__main__":
    import sys
    mode = sys.argv[1] if len(sys.argv) > 1 else "sim"
    sys.path.insert(0, os.path.dirname(os.path.abspath(__file__)))
    import reference

    inputs = {k: np.asarray(v) for k, v in reference.setup_inputs().items()}
    if mode == "sim":
        # Simulate core 0 (b=0, hg=0) with CoreSim and compare to reference.
        # CoreSim has no Gelu; patch exact erf-gelu into its activation visitor.
        import concourse.bass_interp as bass_interp
        from scipy.special import erf as _erf
        _orig_visit = bass_interp.InstructionExecutor.visit_InstActivation

        def _patched(self, instruction, reg_snapshot=None):
            if instruction.func == mybir.ActivationFunctionType.Gelu:
                instruction.func = mybir.ActivationFunctionType.Identity
                try:
                    import concourse.mybir as mb
                    from concourse.bass_interp import Direction
                    out_ap = instruction.outs[0]
                    res = _orig_visit(self, instruction, reg_snapshot=reg_snapshot)
                    v = self.view_ap(out_ap, Direction.WRITE, instruction,
                                     reg_snapshot=reg_snapshot)
                    x = v[:].astype(np.float32)
                    v[:] = (x * 0.5 * (1.0 + _erf(x / np.sqrt(2.0)))).astype(v.dtype)
                    return res
                finally:
                    instruction.func = mybir.ActivationFunctionType.Gelu
            return _orig_visit(self, instruction, reg_snapshot=reg_snapshot)

        bass_interp.InstructionExecutor.visit_InstActivation = _patched
        from concourse.bass_interp import CoreSim
        nc = _get_nc()
        in_maps = _pack_inputs(**inputs)
        sim = CoreSim(nc, trace=False)
        for name, arr in in_maps[0].items():
            sim.tensor(name)[:] = arr
        sim.simulate(check_with_hw=False)
        out = sim.tensor("out").copy()
        got = out.transpose(0, 2, 1, 3).reshape(NS, TG, E)
        exp = np.stack([np.asarray(o) for o in reference.reference(**inputs)])
        exp_slice = exp[:, 0, 0:TG]  # b=0, rows 0:256
        err = np.abs(got - exp_slice)
        rel = np.linalg.norm(got - exp_slice) / np.linalg.norm(exp_slice)
        print(f"max abs err: {err.max():.3e}  rel fro err: {rel:.3e}")
    else:
        got = kernel(**inputs)
        exp = reference.reference(**inputs)
        for n in range(NS):
            g, e = np.asarray(got[n]), np.asarray(exp[n])
            rel = np.linalg.norm(g - e) / np.linalg.norm(e)
            print(f"out{n}: rel fro err {rel:.3e} max abs {np.abs(g - e).max():.3e}")



# revision 44
# speedup vs baseline: 1.3293x; 1.3293x over previous
"""Bass/Trainium2 kernel for nn_BiDirectionalCrossAttentionLayer.

Sharding: 8 cores = batch(4) x head-group(2). Each core computes, for its
batch b and its 4 heads, the full 4-stream cross-attention + the 256 output
rows (t = hg*256 .. hg*256+255) of every stream. The reference's
"transpose(1,2) ... transpose/reshape" scramble maps output row t to
(head t//64, head-dim t%64) over all sequence positions, so a head-split of
attention is exactly an output-row split of everything after it.

All matmuls in bf16 (fp32 accumulate); residuals/LN in fp32.
"""

import os
import numpy as np
import ml_dtypes

import concourse.bacc as bacc
import concourse.bass as bass
import concourse.tile as tile
from concourse import mybir
from concourse.bass_utils import run_bass_kernel_spmd


BF16 = ml_dtypes.bfloat16
F32 = np.float32

NS, B, S, E, H, HD = 4, 4, 512, 512, 8, 64
SCALE = HD ** -0.5
LN_EPS = 1e-5
P = 128
HG = 2            # head groups == cores per batch
HPC = H // HG // 2  # head-pairs per core = 2
HC = H // HG      # heads per core = 4
TG = S // HG      # output rows per core per stream = 256
TS = TG // P      # row tiles per core = 2
ET = E // P       # embedding tiles = 4
KT = S // P       # key/seq tiles = 4
FT = 4 * E // P   # ffn hidden tiles = 16
N_CORES = B * HG

AF = mybir.ActivationFunctionType
ALU = mybir.AluOpType
AX = mybir.AxisListType
DT_BF = mybir.dt.bfloat16
DT_F32 = mybir.dt.float32
DT_I32 = mybir.dt.int32
DT_F8 = mybir.dt.float8e4
F8 = mybir.dt.np(mybir.dt.float8e4)
DR = mybir.MatmulPerfMode.DoubleRow
W_SCALE = 64.0  # host-side W1/W2 prescale: lifts fp8e4 out of subnormals


def _build_program(reps=1, phases="all"):
    nc = bacc.Bacc("TRN2", target_bir_lowering=False, debug=False)

    def din(name, shape, dt=DT_BF):
        return nc.dram_tensor(name, list(shape), dt, kind="ExternalInput").ap()

    xT_d = din("xT", (NS, P, ET, S))            # xT[n,p,et,s] = x[n,b,s,et*128+p]
    x32_d = din("x32", (NS, P, TS, E), DT_F32)  # x rows t-slice
    wq_d = din("wq", (NS, P, ET, HC * HD))      # Wq[n, e, hg*256 + c]
    wk_d = din("wk", (NS, P, ET, HC * HD))
    wv_d = din("wv", (NS, P, ET, HC * HD))
    wo_d = din("wo", (NS, P, ET, E))            # Wo[n]/NS, rows e
    w1_d = din("w1", (NS, P, ET, 4 * E), DT_F8)
    w2_d = din("w2", (NS, P, FT, E), DT_F8)
    cmat_d = din("cmat", (P, NS * NS), DT_F32)  # SCALE*inter broadcast on p
    g1_d = din("g1", (NS, E))
    b1_d = din("b1", (NS, E))
    g2_d = din("g2", (NS, E))
    b2_d = din("b2", (NS, E))
    bf1_d = din("bf1", (NS, 4 * E))             # bf1 row (K=1 matmul operand)
    bf2_d = din("bf2", (NS, E))
    out_d = nc.dram_tensor("out", [NS, P, TS, E], DT_F32, kind="ExternalOutput").ap()

    with tile.TileContext(nc) as tc:
        with tc.tile_pool(name="const", bufs=1) as const:
            cmat_sb = const.tile([P, NS * NS], DT_F32)
            nc.sync.dma_start(cmat_sb[:], cmat_d[:])

            # long-lived activations
            r1 = const.tile([P, NS, TS, E], DT_F32)
            # the xbar DMA transpose emits bf16; the fp8 cast (for the
            # DoubleRow W1 matmul) is a cheap DVE copy per stream
            r1T = const.tile([P, NS, ET, TG], DT_BF)
            r1T8 = const.tile([P, NS, ET, TG], DT_F8)

          # replicated body via HW loop (reps>1 only for slope timing)
          # fmt: off
            import contextlib
            _loop = tc.For_i(0, reps, 1) if reps > 1 else contextlib.nullcontext()
            with _loop:
              # FFN weights (fp8) loaded upfront: their DMAs have no
              # deps, so they prefetch during the DMA-idle attention phase.
              f_w1 = tc.alloc_tile_pool(name="f_w1", bufs=1)
              w1all = f_w1.tile([P, NS, ET, 4 * E], DT_F8)
              scopeB = tc.alloc_tile_pool(name="scopeB", bufs=1)
              x32 = scopeB.tile([P, NS, TS, E], DT_F32)
              att = scopeB.tile([P, NS, KT, HC * HD], DT_BF)  # att_std accum
              nc.gpsimd.memset(att[:], 0.0)

              c_w = tc.alloc_tile_pool(name="c_w", bufs=1)
              g1b = c_w.tile([P, NS, E], DT_BF)
              b1b = c_w.tile([P, NS, E], DT_BF)
              scopeA = tc.alloc_tile_pool(name="scopeA", bufs=1)
              qT = scopeA.tile([P, NS, HPC, S], DT_BF)   # [d-pair rows, n, hp, q]
              kT = scopeA.tile([P, NS, HPC, S], DT_BF)
              # inner dim padded to 80B: DoubleRow ldweights needs the
              # kt-pair stride (HC*inner) and head offset 16B-aligned.
              # cols HD+1..79 are zeroed so attn@v can emit 80 output rows
              # (a 16-multiple, required by the xbar DMA transpose).
              vex = scopeA.tile([P, NS, KT, HC, 80], DT_F8)
              nc.gpsimd.memset(vex[:, :, :, :, HD:HD + 1], 1.0)
              # cols HD+1..79 zeroed -> attn@v emits exact-zero rows 65..79,
              # padding uas to 80 rows (16-multiple for the xbar transpose)
              nc.gpsimd.memset(vex[:, :, :, :, HD + 1:], 0.0)

              # LN1/Wo pools allocated BELOW the attention pools: they stay
              # alive into the FFN region, whose pools then reuse only the
              # attention pools' zones (released right at window end) - not
              # these (a release gated on stream-3's LN would stall the FFN
              # start by ~20us).
              c_sb = tc.alloc_tile_pool(name="c_sb", bufs=1)
              c_wo = tc.alloc_tile_pool(name="c_wo", bufs=2)
              c_sm = tc.alloc_tile_pool(name="c_sm", bufs=4)
              c_rb = tc.alloc_tile_pool(name="c_rb", bufs=1)
              c_ps = tc.alloc_tile_pool(name="c_ps", bufs=2, space="PSUM")

              # Attention pools allocated BEFORE phase-1's so their SBUF/PSUM
              # space is disjoint (no released-zone reuse dep): the j-loop can
              # then overlap the tail of the QKV projections.
              a_sps = tc.alloc_tile_pool(name="a_sps", bufs=2, space="PSUM")
              a_ups = tc.alloc_tile_pool(name="a_ups", bufs=2, space="PSUM")
              a_sb = tc.alloc_tile_pool(name="a_sb", bufs=4)
              a_ub = tc.alloc_tile_pool(name="a_ub", bufs=4)
              a_sm = tc.alloc_tile_pool(name="a_sm", bufs=16)
              a_tr = tc.alloc_tile_pool(name="a_tr", bufs=2)

              # ---------------- Phase 1: QKV projections ----------------
              # Emitted in an order that unblocks attention j=0 early:
              # k0, v0, q0, q1, then k/v for streams 1-3 (these overlap the
              # attention loop). The chain psums share c_ps's two "wops"
              # slots: once phase-1 drains, Wo/LN1 inherits both banks.
              p1w = tc.alloc_tile_pool(name="p1w", bufs=1)
              xTs = p1w.tile([P, NS, ET, S], DT_BF)
              wqs = p1w.tile([P, NS, ET, HC * HD], DT_BF)
              wks = p1w.tile([P, NS, ET, HC * HD], DT_BF)
              wvs = p1w.tile([P, NS, ET, HC * HD], DT_BF)
              # phase-1 inputs first (they gate the first exp), then the
              # LN1 operands that are only needed mid-window
              for n in range(NS):
                  nc.sync.dma_start(xTs[:, n], xT_d[n])
                  nc.sync.dma_start(wqs[:, n], wq_d[n])
                  nc.sync.dma_start(wks[:, n], wk_d[n])
                  nc.sync.dma_start(wvs[:, n], wv_d[n])
              for n in range(NS):
                  nc.sync.dma_start(x32[:, n], x32_d[n])
                  nc.sync.dma_start(g1b[:, n], g1_d[n].partition_broadcast(P))
                  nc.sync.dma_start(b1b[:, n], b1_d[n].partition_broadcast(P))

              def p1_q(n):
                  for hp in range(HPC):
                      ps_q = c_ps.tile([P, S], DT_F32, tag="wops")
                      for et in range(ET):
                          nc.tensor.matmul(
                              ps_q[:], wqs[:, n, et, hp * P:(hp + 1) * P],
                              xTs[:, n, et], start=(et == 0),
                              stop=(et == ET - 1))
                      nc.vector.tensor_copy(qT[:, n, hp], ps_q[:])

              def p1_k(n):
                  for hp in range(HPC):
                      ps_k = c_ps.tile([P, S], DT_F32, tag="wops")
                      for et in range(ET):
                          nc.tensor.matmul(
                              ps_k[:], wks[:, n, et, hp * P:(hp + 1) * P],
                              xTs[:, n, et], start=(et == 0),
                              stop=(et == ET - 1))
                      nc.vector.tensor_copy(kT[:, n, hp], ps_k[:])

              def p1_v(n):
                  for kt in range(KT):
                      ps_v = c_ps.tile([P, S], DT_F32, tag="wops")
                      for et in range(ET):
                          nc.tensor.matmul(
                              ps_v[:, 0:HC * HD],
                              xTs[:, n, et, kt * P:(kt + 1) * P],
                              wvs[:, n, et], start=(et == 0),
                              stop=(et == ET - 1))
                      nc.vector.tensor_copy(
                          vex[:, n, kt, :, 0:HD],
                          ps_v[:, 0:HC * HD].rearrange("p (h d) -> p h d",
                                                       d=HD))

              # g=0 needs only q0/q1 + k/v per j (in order); q2/q3 follow
              p1_k(0), p1_v(0), p1_q(0), p1_q(1)
              for n in range(1, NS):
                  p1_k(n), p1_v(n)
              p1_q(2), p1_q(3)

              # phase-1 weights freed early (zone reused by the FFN pools)
              p1w.release()

              MAGIC = 0x5F3759DF
              K_RSQ_ACT = bool(int(os.environ.get("K_RSQ_ACT", "0")))
              K_NO_TTR = bool(int(os.environ.get("K_NO_TTR", "0")))
              if K_RSQ_ACT:
                  eps_sb = c_w.tile([P, 1], DT_F32)
                  nc.gpsimd.memset(eps_sb[:], LN_EPS)

              def ttr(out, in0, in1, accum_out):
                  # residual add with the mean-reduce fused in one pass.
                  # NOT tensor_tensor_reduce: InstTensorTensorReduce dies
                  # with NRT_EXEC_UNIT_UNRECOVERABLE on this runtime; the
                  # STT accum_out path is the HW-proven equivalent.
                  nc.vector.scalar_tensor_tensor(
                      out=out, in0=in0, scalar=1.0, in1=in1,
                      op0=ALU.mult, op1=ALU.add, accum_out=accum_out)

              def rsqrt_dve(pool, vraw, n, tag):
                  # 1/sqrt(vraw/E + eps) entirely on DVE (bit-trick +
                  # 2 Newton steps, ~5e-6 rel err): no ACT table swaps
                  # inside the exp-saturated attention window.
                  x = pool.tile([P, n], DT_F32, tag=f"{tag}x")
                  y = pool.tile([P, n], DT_F32, tag=f"{tag}y")
                  t = pool.tile([P, n], DT_F32, tag=f"{tag}t")
                  if K_RSQ_ACT:
                      for c in range(n):
                          nc.scalar.activation(y[:, c:c + 1], vraw[:, c:c + 1],
                                               AF.Sqrt, bias=eps_sb[:],
                                               scale=1.0 / E)
                      nc.vector.reciprocal(y[:], y[:])
                      return y
                  nc.vector.tensor_scalar(
                      out=x[:], in0=vraw[:], scalar1=1.0 / E,
                      scalar2=LN_EPS, op0=ALU.mult, op1=ALU.add)
                  nc.vector.tensor_scalar(
                      out=y[:].bitcast(DT_I32), in0=x[:].bitcast(DT_I32),
                      scalar1=1, scalar2=-1,
                      op0=ALU.logical_shift_right, op1=ALU.bitwise_xor)
                  nc.vector.tensor_scalar(
                      out=y[:].bitcast(DT_I32), in0=y[:].bitcast(DT_I32),
                      scalar1=MAGIC + 1, scalar2=None, op0=ALU.add)
                  for _ in range(2):
                      nc.vector.tensor_tensor(
                          out=t[:], in0=y[:], in1=y[:], op=ALU.mult)
                      nc.vector.tensor_tensor(
                          out=t[:], in0=t[:], in1=x[:], op=ALU.mult)
                      nc.vector.tensor_scalar(
                          out=t[:], in0=t[:], scalar1=-0.5, scalar2=1.5,
                          op0=ALU.mult, op1=ALU.add)
                      nc.vector.tensor_tensor(
                          out=y[:], in0=y[:], in1=t[:], op=ALU.mult)
                  return y

              def wo_ln1(i, wo_n, pps, ptag):
                  # Wo proj + residual + LayerNorm1 for stream i, both ts
                  # batched; r1 -> r1T rides the xbar DMA transpose.
                  wo_pss = []
                  for ts in range(TS):
                      wo_ps = pps.tile([P, E], DT_F32, tag=ptag)
                      for qt in range(KT):
                          nc.tensor.matmul(
                              wo_ps[:], att[:, i, qt, ts * P:(ts + 1) * P],
                              wo_n[:, qt], start=(qt == 0),
                              stop=(qt == KT - 1))
                      wo_pss.append(wo_ps)
                  y1 = c_sb.tile([P, TS, E], DT_F32, tag="y1")
                  nm = c_sm.tile([P, TS], DT_F32, tag="nm")
                  for ts in range(TS):
                      # residual add fused with the mean reduction
                      ttr(y1[:, ts], wo_pss[ts][:], x32[:, i, ts],
                          nm[:, ts:ts + 1])
                  nc.vector.tensor_scalar_mul(nm[:], nm[:], -1.0 / E)
                  xc = c_sb.tile([P, TS, E], DT_F32, tag="xc")
                  var = c_sm.tile([P, TS], DT_F32, tag="var")
                  for ts in range(TS):
                      nc.vector.tensor_scalar_add(xc[:, ts], y1[:, ts],
                                                  nm[:, ts:ts + 1])
                      # y1 reused as the squared scratch
                      nc.vector.scalar_tensor_tensor(
                          out=y1[:, ts], in0=xc[:, ts], scalar=1.0,
                          in1=xc[:, ts], op0=ALU.mult, op1=ALU.mult,
                          accum_out=var[:, ts:ts + 1])
                  inv = rsqrt_dve(c_sm, var, TS, "l1")
                  for ts in range(TS):
                      nc.vector.scalar_tensor_tensor(
                          out=r1[:, i, ts], in0=xc[:, ts],
                          scalar=inv[:, ts:ts + 1],
                          in1=g1b[:, i], op0=ALU.mult, op1=ALU.mult)
                      nc.vector.tensor_add(r1[:, i, ts], r1[:, i, ts],
                                           b1b[:, i])
                  r1b = c_rb.tile([P, TS, E], DT_BF, tag="r1b")
                  nc.vector.tensor_copy(r1b[:], r1[:, i])
                  for ts in range(TS):
                      nc.sync.dma_start(r1T[:, i, :, ts * P:(ts + 1) * P],
                                        r1b[:, ts], transpose=True)
                  nc.vector.tensor_copy(
                      r1T8[:, i].rearrange("p a b -> p (a b)"),
                      r1T[:, i].rearrange("p a b -> p (a b)"))

              # ---------------- Phase 2: cross-stream attention ----------
              # Heads of a pair live on disjoint PE row strips (partitions
              # 0-63 / 64-127): issuing their score matmuls back-to-back
              # lets the PE run them concurrently. g-outer/j-inner: a query
              # pair's att finishes at its g-loop end, so its Wo+LN1 (PE/
              # DVE only - no ACT) hides under the next pair's exp stream.
              if True:
                  # j-inner: per key-stream j, do scores+exp for a pair of
                  # query-streams g={i0,i1}, then attn@v with the vex
                  # stationary shared by consecutive matmuls (ldweights
                  # elision). The [d,q]->[q,d] flip of the attn@v outputs
                  # rides the xbar DMA-transpose engine (idle) instead of
                  # the PE: evac psum->sbuf once, DMA-transpose per i2 into
                  # trT[q, i2, qt, hl, d], then one batched reciprocal and
                  # one fused accumulate per (i, qt).
                  attn_on = phases in ("all", "attn", "mmx", "mm2")
                  # "woln": attention+FFN off, wo_ln1 only
                  act_on = phases in ("all", "attn")
                  norm_on = phases != "mm2"
                  full_on = phases in ("all", "noattn")
                  wo_tiles = {}

                  def load_wo(i):
                      t = c_wo.tile([P, ET, E], DT_BF, tag="won")
                      nc.gpsimd.dma_start(t[:], wo_d[i])
                      wo_tiles[i] = t

                  if full_on:
                      load_wo(0), load_wo(1)
                  for g in range(2 if attn_on else 0):
                      for j in range(NS):
                          trT = a_tr.tile([P, 2, KT, HC, 80], DT_BF,
                                          tag="trT")
                          for hp in range(HPC):
                              exs = []  # per i2: ex tile [P, 2, KT, S]
                              for i2 in range(2):
                                  i = g * 2 + i2
                                  c_ap = cmat_sb[:,
                                                 (i * NS + j):(i * NS + j + 1)]
                                  ex = a_sb.tile([P, 2, KT, S], DT_F8,
                                                 tag="ex")
                                  for kt in range(KT):
                                      s = a_sps.tile([P, 2, S], DT_F32,
                                                     tag="s")
                                      nc.tensor.matmul(
                                          s[:, 0],
                                          kT[0:HD, j, hp, kt * P:(kt + 1) * P],
                                          qT[0:HD, i, hp],
                                          start=True, stop=True)
                                      nc.tensor.matmul(
                                          s[:, 1],
                                          kT[HD:P, j, hp, kt * P:(kt + 1) * P],
                                          qT[HD:P, i, hp],
                                          start=True, stop=True)
                                      if act_on:
                                          nc.scalar.activation(
                                              ex[:, :, kt, :], s[:], AF.Exp,
                                              scale=c_ap)
                                      else:
                                          # timing knockout: keep the psum
                                          # live with a tiny ACT read
                                          nc.scalar.activation(
                                              ex[:, :, kt, 0:4], s[:, :, 0:4],
                                              AF.Exp, scale=c_ap)
                                  exs.append(ex)
                              for sub in range(2):
                                  hl = hp * 2 + sub
                                  uas = [a_ups.tile([80, S], DT_F32,
                                                    tag="ua", name=f"ua{i2}")
                                         for i2 in range(2)]
                                  for ktp in range(KT // 2):
                                      # same vex stationary for both i2 mms
                                      for i2 in range(2):
                                          nc.tensor.matmul(
                                              uas[i2][:],
                                              vex[:, j,
                                                  2 * ktp:2 * ktp + 2, hl,
                                                  0:80],
                                              exs[i2][:, sub,
                                                      2 * ktp:2 * ktp + 2],
                                              start=(ktp == 0),
                                              stop=(ktp == KT // 2 - 1),
                                              perf_mode=DR)
                                  usb = a_ub.tile([80, 2, S], DT_BF,
                                                  tag="usb")
                                  for i2 in range(2):
                                      nc.vector.tensor_copy(usb[:, i2],
                                                            uas[i2][:])
                                      # issue on SP: a transpose trigger
                                      # waiting on the usb copy must not
                                      # block exp ACTIVATEs in the ACT FIFO
                                      nc.sync.dma_start(
                                          trT[:, i2, :, hl, :], usb[:, i2],
                                          transpose=True)
                          for i2 in range(2 if norm_on else 0):
                              i = g * 2 + i2
                              rr = a_sm.tile([P, KT], DT_F32, tag="rr")
                              nc.vector.reciprocal(
                                  rr[:], trT[:, i2, :, 0, HD:HD + 1])
                              for qt in range(KT):
                                  att_ap = att[:, i, qt, :].rearrange(
                                      "p (h d) -> p h d", h=HC)
                                  nc.vector.scalar_tensor_tensor(
                                      out=att_ap,
                                      in0=trT[:, i2, qt, :, 0:HD],
                                      scalar=rr[:, qt:qt + 1],
                                      in1=att_ap,
                                      op0=ALU.mult, op1=ALU.add)
                          if not norm_on:
                              # keep trT live: fold into att via add
                              nc.vector.tensor_add(
                                  att[:, g].rearrange(
                                      "p a (h d) -> p a h d", h=HC),
                                  att[:, g].rearrange(
                                      "p a (h d) -> p a h d", h=HC),
                                  trT[:, 0, :, :, 0:HD])
                          # trickle one FFN W1 stream load per 2 iterations.
                          # SWDGE (gpsimd): a 1MB load on the SP HWDGE ring
                          # would stall the trT transposes queued behind it
                          # (-> 11us pipeline bubble).
                          idx = g * NS + j
                          if idx % 2 == 0:
                              nc.gpsimd.dma_start(w1all[:, idx // 2],
                                                  w1_d[idx // 2])
                      # this pair's Wo+LN1 hides under the next pair's exps
                      # (g=1's pair is emitted inside the FFN block so the
                      # FFN pool allocs don't wait on it)
                      for i2 in range(2 if (full_on and g == 0) else 0):
                          i = g * 2 + i2
                          wo_ln1(i, wo_tiles.pop(i), c_ps, "wops")
                          load_wo(i + 2)

              if not attn_on and phases in ("noattn", "woln"):
                  # attention loop skipped: load FFN weights here instead
                  for n in range(NS):
                      nc.sync.dma_start(w1all[:, n], w1_d[n])
                  load_wo(0), load_wo(1), load_wo(2), load_wo(3)

              if phases == "qkv":
                  # keep q/k/v live so DCE cannot drop the projections
                  with tc.tile_pool(name="qo", bufs=2) as qo:
                      for n in range(NS):
                          qout = qo.tile([P, TS, E], DT_F32, tag="qout")
                          nc.vector.tensor_copy(
                              qout[:].rearrange("p a b -> p (a b)"),
                              qT[:, n].rearrange("p a b -> p (a b)"))
                          nc.vector.tensor_add(
                              qout[:].rearrange("p a b -> p (a b)"),
                              qout[:].rearrange("p a b -> p (a b)"),
                              kT[:, n].rearrange("p a b -> p (a b)"))
                          nc.vector.tensor_add(
                              qout[:, 0, 0:256].rearrange(
                                  "p (a c) -> p a c", a=KT),
                              qout[:, 0, 0:256].rearrange(
                                  "p (a c) -> p a c", a=KT),
                              vex[:, n, :, 0, 0:HD])
                          nc.sync.dma_start(out_d[n], qout[:])

              a_tr.release(), a_sm.release(), a_ub.release(), a_sb.release()
              a_ups.release(), a_sps.release()
              c_ps.release()

              if phases in ("attn", "mmx", "mm2"):
                  # copy att into the output so DCE cannot drop the attention
                  with tc.tile_pool(name="ao", bufs=2) as ao:
                      for n in range(NS):
                          aout = ao.tile([P, TS, E], DT_F32, tag="aout")
                          nc.vector.tensor_copy(
                              aout[:].rearrange("p a b -> p (a b)"),
                              att[:, n].rearrange("p a b -> p (a b)"))
                          nc.sync.dma_start(out_d[n], aout[:])

              # ------- Phase 4+5: per-stream FFN, software-pipelined ------
              # Emission order W1(0),W1(1),W2(0),W1(2),W2(1),... keeps the
              # PE FIFO busy while gelu(n) runs: W2(n) at the queue head
              # would otherwise stall W1(n+1) behind it.
              f_c = tc.alloc_tile_pool(name="f_c", bufs=1)
              bf1r = f_c.tile([1, NS, 4 * E], DT_BF)
              ones_row = f_c.tile([1, TG], DT_BF)
              nc.gpsimd.memset(ones_row[:], 1.0)
              g2b = f_c.tile([P, NS, E], DT_BF)
              b2b = f_c.tile([P, NS, E], DT_BF)
              bf2b = f_c.tile([P, NS, E], DT_BF)
              nc.sync.dma_start(bf1r[:], bf1_d[None, :, :])
              for n in range(NS):
                  nc.sync.dma_start(g2b[:, n], g2_d[n].partition_broadcast(P))
                  nc.sync.dma_start(b2b[:, n], b2_d[n].partition_broadcast(P))
                  nc.sync.dma_start(bf2b[:, n], bf2_d[n].partition_broadcast(P))

              with tc.tile_pool(name="f_ps", bufs=2, space="PSUM") as f_ps, \
                   tc.tile_pool(name="f_ps2", bufs=3, space="PSUM") as f_ps2, \
                   tc.tile_pool(name="f_hT", bufs=2) as f_hT, \
                   tc.tile_pool(name="f_sb", bufs=2) as f_sb, \
                   tc.tile_pool(name="f_sb2", bufs=1) as f_sb2, \
                   tc.tile_pool(name="f_w2", bufs=2) as f_w2, \
                   tc.tile_pool(name="f_sm", bufs=4) as f_sm:
                  hTs, w2ss = {}, {}

                  def ffn_w1(n):
                      # W2 weights prefetch (SWDGE) in parallel with W1
                      w2s = f_w2.tile([P, FT, E], DT_F8, tag="w2s")
                      nc.gpsimd.dma_start(w2s[:], w2_d[n])
                      w2ss[n] = w2s
                      hT = f_hT.tile([P, FT, TG], DT_F8, tag="hT")
                      hTs[n] = hT
                      # bf1 enters via a K=1 ones-row matmul so gelu can
                      # batch 4 hidden slices per ACTIVATE with no
                      # per-slice bias.
                      for f4 in range(FT // 4):
                          h_ps = f_ps.tile([P, 4, TG], DT_F32, tag="hps")
                          for s4 in range(4):
                              fs = f4 * 4 + s4
                              for etp in range(ET // 2):
                                  nc.tensor.matmul(
                                      h_ps[:, s4],
                                      w1all[:, n, 2 * etp:2 * etp + 2,
                                            fs * P:(fs + 1) * P],
                                      r1T8[:, n, 2 * etp:2 * etp + 2],
                                      start=(etp == 0), stop=False,
                                      perf_mode=DR)
                              nc.tensor.matmul(
                                  h_ps[:, s4],
                                  bf1r[0:1, n, fs * P:(fs + 1) * P],
                                  ones_row[:], start=False, stop=True)
                          # psum holds W_SCALE*(r1@W1 + bf1)
                          nc.scalar.activation(hT[:, f4 * 4:(f4 + 1) * 4],
                                               h_ps[:], AF.Gelu,
                                               scale=1.0 / W_SCALE)

                  def ffn_w2(n):
                      hT, w2s = hTs.pop(n), w2ss.pop(n)
                      out_sb = f_sb.tile([P, TS, E], DT_F32, tag="outsb")
                      y2 = f_sb2.tile([P, TS, E], DT_F32, tag="y2")
                      nm = f_sm.tile([P, TS], DT_F32, tag="nm2")
                      xc = f_sb2.tile([P, TS, E], DT_F32, tag="xc2")
                      var = f_sm.tile([P, TS], DT_F32, tag="var2")
                      tmp = f_sb2.tile([P, E], DT_F32, tag="tmp2")
                      for ts in range(TS):
                          f2_ps = f_ps2.tile([P, E], DT_F32, tag="fps")
                          for ftp in range(FT // 2):
                              nc.tensor.matmul(
                                  f2_ps[:],
                                  hT[:, 2 * ftp:2 * ftp + 2,
                                     ts * P:(ts + 1) * P],
                                  w2s[:, 2 * ftp:2 * ftp + 2],
                                  start=(ftp == 0), stop=(ftp == FT // 2 - 1),
                                  perf_mode=DR)
                          # psum holds W_SCALE*(h@W2): scale down + bias
                          nc.vector.scalar_tensor_tensor(
                              out=tmp[:], in0=f2_ps[:], scalar=1.0 / W_SCALE,
                              in1=bf2b[:, n], op0=ALU.mult, op1=ALU.add)
                          # residual add fused with the mean reduction
                          ttr(y2[:, ts], tmp[:], r1[:, n, ts],
                              nm[:, ts:ts + 1])
                      nc.vector.tensor_scalar_mul(nm[:], nm[:], -1.0 / E)
                      for ts in range(TS):
                          nc.vector.tensor_scalar_add(xc[:, ts], y2[:, ts],
                                                      nm[:, ts:ts + 1])
                          # y2 reused as the squared scratch
                          nc.vector.scalar_tensor_tensor(
                              out=y2[:, ts], in0=xc[:, ts], scalar=1.0,
                              in1=xc[:, ts], op0=ALU.mult, op1=ALU.mult,
                              accum_out=var[:, ts:ts + 1])
                      inv = rsqrt_dve(f_sm, var, TS, "l2")
                      for ts in range(TS):
                          nc.vector.scalar_tensor_tensor(
                              out=out_sb[:, ts], in0=xc[:, ts],
                              scalar=inv[:, ts:ts + 1],
                              in1=g2b[:, n], op0=ALU.mult, op1=ALU.mult)
                          nc.vector.tensor_add(out_sb[:, ts], out_sb[:, ts],
                                               b2b[:, n])
                          nc.sync.dma_start(out_d[n, :, ts], out_sb[:, ts])

                  if phases == "woln":
                      # knockout: wo_ln1 only, r1 copied out
                      for i in range(NS):
                          wo_ln1(i, wo_tiles.pop(i), f_ps2, "fps")
                      with tc.tile_pool(name="wo_o", bufs=2) as wo_o:
                          for n in range(NS):
                              ro = wo_o.tile([P, TS, E], DT_F32, tag="ro")
                              nc.vector.tensor_copy(
                                  ro[:].rearrange("p a b -> p (a b)"),
                                  r1[:, n].rearrange("p a b -> p (a b)"))
                              nc.sync.dma_start(out_d[n], ro[:])
                  if full_on:
                      # W1(0) first: it only needs r1T8(0) (ready mid-
                      # window), so the PE takes it the moment the window
                      # drains; streams 2/3's Wo+LN1 interleave after.
                      ffn_w1(0)
                      if attn_on:
                          wo_ln1(2, wo_tiles.pop(2), f_ps2, "fps")
                      ffn_w1(1)
                      if attn_on:
                          wo_ln1(3, wo_tiles.pop(3), f_ps2, "fps")
                      else:
                          for i in range(NS):
                              wo_ln1(i, wo_tiles.pop(i), f_ps2, "fps")
                      ffn_w2(0)
                      ffn_w1(2)
                      ffn_w2(1)
                      ffn_w1(3)
                      ffn_w2(2)
                      ffn_w2(3)
              f_c.release()
              c_rb.release(), c_sm.release(), c_wo.release(), c_sb.release()
              scopeA.release(), c_w.release(), scopeB.release()
              f_w1.release()

    nc.compile()
    return nc


_NC_CACHE = {}


def _get_nc(reps=1, phases="all"):
    key = f"nc{reps}_{phases}"
    if key not in _NC_CACHE:
        _NC_CACHE[key] = _build_program(reps, phases)
    return _NC_CACHE[key]


def _pack_inputs(x0, x1, x2, x3, Wq, Wk, Wv, Wo, bo, ln1_g, ln1_b, ln2_g, ln2_b,
                 W1, bf1, W2, bf2, inter):
    x = np.stack([np.asarray(x0), np.asarray(x1), np.asarray(x2),
                  np.asarray(x3)]).astype(F32)  # [NS,B,S,E]
    Wq, Wk, Wv, Wo = (np.asarray(a, F32) for a in (Wq, Wk, Wv, Wo))
    inputs_bo = np.asarray(bo, F32)
    W1, W2 = np.asarray(W1, F32), np.asarray(W2, F32)
    inter = np.asarray(inter, F32)

    def tile_rows(a, nt):
        # [NS, R, C] -> [NS, P, nt, C]
        return np.ascontiguousarray(
            a.reshape(NS, nt, P, a.shape[-1]).transpose(0, 2, 1, 3))

    shared = {
        "wo": tile_rows(Wo / NS, ET).astype(BF16),
        "w1": tile_rows(W1 * W_SCALE, ET).astype(F8),
        "w2": tile_rows(W2 * W_SCALE, FT).astype(F8),
        "cmat": np.ascontiguousarray(
            np.broadcast_to((inter * SCALE).reshape(1, NS * NS), (P, NS * NS))
        ).astype(F32),
        "g1": np.ascontiguousarray(ln1_g).astype(BF16),
        "b1": np.ascontiguousarray(ln1_b).astype(BF16),
        "g2": np.ascontiguousarray(ln2_g).astype(BF16),
        "b2": np.ascontiguousarray(ln2_b).astype(BF16),
        "bf1": np.ascontiguousarray(np.asarray(bf1, F32) * W_SCALE).astype(BF16),
        "bf2": np.ascontiguousarray(bf2).astype(BF16),
    }
    per_hg = []
    for hg in range(HG):
        cols = slice(hg * HC * HD, (hg + 1) * HC * HD)
        per_hg.append({
            "wq": tile_rows(Wq[:, :, cols], ET).astype(BF16),
            "wk": tile_rows(Wk[:, :, cols], ET).astype(BF16),
            "wv": tile_rows(Wv[:, :, cols], ET).astype(BF16),
        })
    in_maps = []
    for core in range(N_CORES):
        b, hg = core // HG, core % HG
        xb = x[:, b]  # [NS, S, E]
        xT = np.ascontiguousarray(
            xb.transpose(0, 2, 1).reshape(NS, ET, P, S).transpose(0, 2, 1, 3)
        ).astype(BF16)
        x32 = np.ascontiguousarray(
            (xb[:, hg * TG:(hg + 1) * TG] + np.asarray(
                inputs_bo)[:, None, :]).reshape(NS, TS, P, E)
            .transpose(0, 2, 1, 3).astype(F32))
        m = {"xT": xT, "x32": x32}
        m.update(shared)
        m.update(per_hg[hg])
        in_maps.append(m)
    return in_maps


def _unpack_outputs(results):
    full = np.empty((NS, B, S, E), dtype=F32)
    for core in range(N_CORES):
        b, hg = core // HG, core % HG
        o = results[core]["out"]  # [NS, P, TS, E]
        full[:, b, hg * TG:(hg + 1) * TG] = (
            o.transpose(0, 2, 1, 3).reshape(NS, TG, E))
    return tuple(full[n] for n in range(NS))


def kernel(**inputs):
    nc = _get_nc()
    in_maps = _pack_inputs(**inputs)
    res = run_bass_kernel_spmd(
        nc, in_maps, core_ids=list(range(N_CORES)),
        trace=bool(int(os.environ.get("KERNEL_TRACE", "0"))))
    _NC_CACHE["last_result"] = res
    return _unpack_outputs(res.results)


def _make_bench_fn(inputs, reps=1, phases="all"):
    """Build a jitted on-device executable for the kernel with `reps`
    replications of the body (HW loop). Returns (fn, concat_args)."""
    import jax
    import jax.numpy as jnp
    from jax.sharding import Mesh, PartitionSpec, NamedSharding
    from jax.experimental.shard_map import shard_map
    from concourse import bass2jax
    from concourse import mybir as mb

    nc = _get_nc(reps, phases)
    bass2jax.install_neuronx_cc_hook()
    in_maps = _pack_inputs(**inputs)

    part_name = nc.partition_id_tensor.name if nc.partition_id_tensor else None
    in_names, out_names, out_avals, zero_outs = [], [], [], []
    for alloc in nc.m.functions[0].allocations:
        if not isinstance(alloc, mb.MemoryLocationSet):
            continue
        name = alloc.memorylocations[0].name
        if alloc.kind == "ExternalInput":
            if name != part_name:
                in_names.append(name)
        elif alloc.kind == "ExternalOutput":
            out_names.append(name)
            shape = tuple(alloc.tensor_shape)
            dtype = mb.dt.np(alloc.dtype)
            out_avals.append(jax.core.ShapedArray(shape, dtype))
            zero_outs.append(np.zeros(shape, dtype))
    n_params = len(in_names)
    all_names = in_names + out_names
    if part_name is not None:
        all_names = all_names + [part_name]

    def _body(*args):
        operands = list(args)
        if part_name is not None:
            operands.append(bass2jax.partition_id_tensor())
        outs = bass2jax._bass_exec_p.bind(
            *operands, out_avals=tuple(out_avals), in_names=tuple(all_names),
            out_names=tuple(out_names), lowering_input_output_aliases=(),
            sim_require_finite=True, sim_require_nnan=True, nc=nc)
        return tuple(outs)

    devices = jax.devices()[:N_CORES]
    mesh = Mesh(np.asarray(devices), ("core",))
    spec = PartitionSpec("core")
    fn = jax.jit(shard_map(
        _body, mesh=mesh, in_specs=(spec,) * (n_params + len(out_names)),
        out_specs=(spec,) * len(out_names), check_rep=False))
    sh = NamedSharding(mesh, spec)
    concat = [jax.device_put(
        np.concatenate([in_maps[c][nm] for c in range(N_CORES)], axis=0), sh)
        for nm in in_names]
    concat += [jax.device_put(
        np.zeros((N_CORES * z.shape[0], *z.shape[1:]), z.dtype), sh)
        for z in zero_outs]

    out = fn(*concat)  # compile
    jax.block_until_ready(out)
    return fn, concat


def _time_fn(fn, concat):
    import time
    import jax
    t0 = time.perf_counter()
    out = fn(*concat)
    jax.block_until_ready(out)
    return time.perf_counter() - t0


def bench(inputs, iters=20, reps=1, phases="all"):
    """Time the on-device execution with device-resident inputs.
    Returns (min, median) seconds per call."""
    fn, concat = _make_bench_fn(inputs, reps, phases)
    times = sorted(_time_fn(fn, concat) for _ in range(iters))
    return times[0], times[len(times) // 2]


def bench_paired(inputs, iters=30, reps_hi=33, phases="all", phases_lo=None):
    """Robust per-rep device time: interleave a 1-rep and a reps_hi-rep
    executable; per-round difference cancels RPC/driver drift. Returns
    (median_slope_s, min_slope_s, raw_diffs)."""
    fn1, c1 = _make_bench_fn(inputs, 1, phases_lo or phases)
    fnH, cH = _make_bench_fn(inputs, reps_hi, phases)
    # warm both
    for _ in range(3):
        _time_fn(fn1, c1), _time_fn(fnH, cH)
    diffs = []
    for _ in range(iters):
        t1 = _time_fn(fn1, c1)
        tH = _time_fn(fnH, cH)
        t1b = _time_fn(fn1, c1)
        diffs.append(tH - min(t1, t1b))
    diffs.sort()
    n = reps_hi - 1
    return diffs[len(diffs) // 2] / n, diffs[0] / n, diffs


if __name__ == "__main__":
    import sys
    mode = sys.argv[1] if len(sys.argv) > 1 else "sim"
    sys.path.insert(0, os.path.dirname(os.path.abspath(__file__)))
    import reference

    inputs = {k: np.asarray(v) for k, v in reference.setup_inputs().items()}
    if mode == "sim":
        # Simulate core 0 (b=0, hg=0) with CoreSim and compare to reference.
        # CoreSim has no Gelu; patch exact erf-gelu into its activation visitor.
        import concourse.bass_interp as bass_interp
        from scipy.special import erf as _erf
        _orig_visit = bass_interp.InstructionExecutor.visit_InstActivation

        def _patched(self, instruction, reg_snapshot=None):
            if instruction.func == mybir.ActivationFunctionType.Gelu:
                instruction.func = mybir.ActivationFunctionType.Identity
                try:
                    import concourse.mybir as mb
                    from concourse.bass_interp import Direction
                    out_ap = instruction.outs[0]
                    res = _orig_visit(self, instruction, reg_snapshot=reg_snapshot)
                    v = self.view_ap(out_ap, Direction.WRITE, instruction,
                                     reg_snapshot=reg_snapshot)
                    x = v[:].astype(np.float32)
                    v[:] = (x * 0.5 * (1.0 + _erf(x / np.sqrt(2.0)))).astype(v.dtype)
                    return res
                finally:
                    instruction.func = mybir.ActivationFunctionType.Gelu
            return _orig_visit(self, instruction, reg_snapshot=reg_snapshot)

        bass_interp.InstructionExecutor.visit_InstActivation = _patched
        from concourse.bass_interp import CoreSim
        nc = _get_nc()
        in_maps = _pack_inputs(**inputs)
        sim = CoreSim(nc, trace=False)
        for name, arr in in_maps[0].items():
            sim.tensor(name)[:] = arr
        sim.simulate(check_with_hw=False)
        out = sim.tensor("out").copy()
        got = out.transpose(0, 2, 1, 3).reshape(NS, TG, E)
        exp = np.stack([np.asarray(o) for o in reference.reference(**inputs)])
        exp_slice = exp[:, 0, 0:TG]  # b=0, rows 0:256
        err = np.abs(got - exp_slice)
        rel = np.linalg.norm(got - exp_slice) / np.linalg.norm(exp_slice)
        print(f"max abs err: {err.max():.3e}  rel fro err: {rel:.3e}")
    else:
        got = kernel(**inputs)
        exp = reference.reference(**inputs)
        for n in range(NS):
            g, e = np.asarray(got[n]), np.asarray(exp[n])
            rel = np.linalg.norm(g - e) / np.linalg.norm(e)
            print(f"out{n}: rel fro err {rel:.3e} max abs {np.abs(g - e).max():.3e}")

